# revision 3
# baseline (speedup 1.0000x reference)
"""Banded-attention (AttentionWindow) TRN2 kernel, data-parallel over batch on 8 NeuronCores.

Reference computation (per batch b):
  Q = x @ W;  scores = Q @ x^T;  scores[|i-j| > 64] = -1e9
  probs = softmax(scores, axis=-1);  out = x + relu(probs @ x)

Kernel strategy (v2):
  - One batch per core (batch=8, n_cores=8), W replicated. No collectives.
  - Everything on the matmul path is bf16 (validated offline: rel_fro ~8e-3
    vs the 2e-2 gate): x, W, Q, probs, PV operands, the residual add and the
    output DMA. Softmax statistics (max/sum/recip) and PSUM stay fp32.
    bf16 halves DMA bytes — the baseline was DMA-line-rate bound — and
    enables fast PE weight loads.
  - The |i-j| <= 64 band means each 128-query tile attends to an exact
    256-wide key window at kx = clamp(128i-64, 0, 1792). Interior windows
    are 64-shifted, so PV uses 64-shifted copies of x (xs tiles, DMA'd
    straight from the same xn dram tensor at row offset 64) and needs only
    2 key chunks -- no padded 384-wide probs, no memsets.
  - PE warm-up uses REAL matmuls on memset tiles (PE-transposes do not
    engage the HAM activity monitor, so the old transpose warm-up left the
    clock throttled at 1.2 GHz for the first ~22 us).
  - Tail is split: head(i) | transpose(i-1) | PV+relu+add(i-2), so the
    PSUM->SBUF copy of transposed probs never stalls the PE.
  - DMA: per-partition line rate is the scarce resource; xn/xs pairs load
    as single strided 2-tile DMAs (4KB 2-burst lines), xt bulk as 3KB
    lines, and loads are ordered so w+xt-group0 land first.

Inputs: repr [8, 2048, 1024] f32, W [1024, 1024] f32.
Output: [8, 2048, 1024] f32.
"""
from contextlib import ExitStack

import numpy as np

SEQ, HID = 2048, 1024
W2 = 64                  # window half-width
QTL = 128                # queries per softmax tile
KX = 256                 # exact key window per q-tile
NQ = SEQ // QTL          # 16
GQ = 512                 # queries per Q^T-projection group
NG = SEQ // GQ           # 4
ND = HID // 128          # 8 contraction chunks
NEG = -1e9
WARM_N = 96              # PE warm-up matmuls (fills the initial DMA wait)


def _legalize_waits(nc):
    """This walrus build accepts 1 sync wait per instruction (2 on
    EventSemaphore). Hoist excess waits onto EventSemaphore prefixes on the
    same engine."""
    from concourse import mybir

    n = 0
    for func in nc.m.functions:
        for blk in func.blocks:
            out = []
            changed = False
            for inst in list(blk.instructions):
                si = inst.sync_info
                cap = 2 if isinstance(inst, mybir.InstEventSemaphore) else 1
                if si is not None and len(si.on_wait) > cap:
                    waits = list(si.on_wait)
                    for i in range(cap, len(waits), 2):
                        ev = mybir.InstEventSemaphore(
                            name=f"{inst.name}_waitfix{i}",
                            engine=inst.engine,
                            ins=[],
                            outs=[],
                            sync_info=mybir.SyncInfo(on_wait=waits[i:i + 2],
                                                     on_update=[]),
                        )
                        out.append(ev)
                        n += 1
                    inst.sync_info = mybir.SyncInfo(on_wait=waits[:cap],
                                                    on_update=list(si.on_update))
                    changed = True
                out.append(inst)
            if changed:
                blk.instructions = out
    return n


def _build(nc):
    import concourse.tile as tile
    from concourse import masks, mybir

    F32 = mybir.dt.float32
    BF16 = mybir.dt.bfloat16
    AF = mybir.ActivationFunctionType
    ALU = mybir.AluOpType
    X = mybir.AxisListType.X

    w = nc.dram_tensor("w", [HID, HID], BF16, kind="ExternalInput").ap()
    xt = nc.dram_tensor("xt", [HID, SEQ], BF16, kind="ExternalInput").ap()
    xn = nc.dram_tensor("xn", [SEQ, HID], BF16, kind="ExternalInput").ap()
    out = nc.dram_tensor("out", [SEQ, HID], BF16, kind="ExternalOutput").ap()

    with tile.TileContext(nc) as tc, ExitStack() as ctx:
        pool = ctx.enter_context(tc.tile_pool(name="sb", bufs=1))
        ps = ctx.enter_context(tc.tile_pool(name="ps", bufs=1, space="PSUM"))

        # ---- warm-up tiles (no DMA deps): memset first so the PE can start
        # real matmuls immediately and warm the HAM clock gate.
        warm_w = pool.tile([128, 128], BF16, tag="warmw", name="warmw")
        warm_x = pool.tile([128, 512], BF16, tag="warmx", name="warmx")
        nc.gpsimd.memset(warm_w[:], 0.0)
        nc.gpsimd.memset(warm_x[:], 0.0)

        pq0 = ps.tile([128, GQ], F32, tag="q0", name="warmps0")
        pq1 = ps.tile([128, GQ], F32, tag="q1", name="warmps1")
        for k in range(WARM_N):
            t = pq0 if k % 2 == 0 else pq1
            nc.tensor.matmul(t[:], warm_w[:], warm_x[:], start=True, stop=True)

        # ---- resident inputs
        wt = [pool.tile([128, HID], BF16, tag=f"w{d}", name=f"w{d}") for d in range(ND)]
        xtt = [pool.tile([128, SEQ], BF16, tag=f"xt{d}", name=f"xt{d}") for d in range(ND)]
        # xn tile pairs: xnt2[m] cols [0:1024] = seq tile 2m, [1024:2048] = 2m+1
        xnt2 = [pool.tile([128, 2 * HID], BF16, tag=f"xn{m}", name=f"xn{m}")
                for m in range(NQ // 2)]
        # 64-shifted x tiles for PV (interior windows); xst2[m] = xs tiles 2m, 2m+1
        xst2 = [pool.tile([128, 2 * HID], BF16, tag=f"xs{m}", name=f"xs{m}")
                for m in range(7)]
        xs14 = pool.tile([128, HID], BF16, tag="xs14", name="xs14")

        def xn_ap(k):
            return xnt2[k // 2][:, (k % 2) * HID:(k % 2 + 1) * HID]

        def xs_ap(j):
            if j == 14:
                return xs14[:]
            return xst2[j // 2][:, (j % 2) * HID:(j % 2 + 1) * HID]

        def dma_pair(dst_tile, src_rows):
            """One strided DMA moving two 128-row seq tiles (dram rows
            [src_rows, src_rows+256)) into a [128, 2048] SBUF tile, split
            into two 64-partition DMAs for queue spread."""
            src = xn[src_rows:src_rows + 256, :].rearrange("(a b) c -> b a c", a=2)
            dst = dst_tile[:].rearrange("p (a c) -> p a c", a=2)
            nc.sync.dma_start(dst[0:64], src[0:64])
            nc.sync.dma_start(dst[64:128], src[64:128])

        # ---- input DMAs, ordered by first use; [64p] splits spread queues.
        for d in range(ND):          # w lo half: needed by proj e=0..3
            nc.sync.dma_start(wt[d][0:64, 0:512], w[128 * d:128 * d + 64, 0:512])
            nc.sync.dma_start(wt[d][64:128, 0:512], w[128 * d + 64:128 * (d + 1), 0:512])
        for d in range(ND):          # xt group 0: queries 0:512
            nc.sync.dma_start(xtt[d][0:64, 0:GQ], xt[128 * d:128 * d + 64, 0:GQ])
            nc.sync.dma_start(xtt[d][64:128, 0:GQ], xt[128 * d + 64:128 * (d + 1), 0:GQ])
        for d in range(ND):          # w hi half: proj e=4..7
            nc.sync.dma_start(wt[d][0:64, 512:HID], w[128 * d:128 * d + 64, 512:HID])
            nc.sync.dma_start(wt[d][64:128, 512:HID], w[128 * d + 64:128 * (d + 1), 512:HID])
        dma_pair(xnt2[0], 0)         # xnt 0,1: tile-0 PV + first residuals
        dma_pair(xst2[0], 64)        # xs 0,1: tile-1/2 PV
        for d in range(ND):          # xt remainder: 3KB lines
            nc.sync.dma_start(xtt[d][0:64, GQ:SEQ], xt[128 * d:128 * d + 64, GQ:SEQ])
            nc.sync.dma_start(xtt[d][64:128, GQ:SEQ], xt[128 * d + 64:128 * (d + 1), GQ:SEQ])
        dma_pair(xnt2[1], 256)
        dma_pair(xst2[1], 64 + 256)
        for m in range(2, 7):        # interleave by need time
            dma_pair(xnt2[m], 256 * m)
            dma_pair(xst2[m], 64 + 256 * m)
        dma_pair(xnt2[7], 256 * 7)
        nc.sync.dma_start(xs14[0:64, :], xn[64 + 128 * 14:64 + 128 * 14 + 64, :])
        nc.sync.dma_start(xs14[64:128, :], xn[64 + 128 * 14 + 64:64 + 128 * 15, :])

        # ---- identity (bf16, for probs transposes) + banded masks
        idn = pool.tile([128, 128], BF16, tag="idn", name="idn")
        masks.make_identity(nc, idn[:])
        mask_by_off = {}
        for off in (0, 64, 128):
            m = pool.tile([128, KX], F32, tag=f"mask{off}", name=f"mask{off}")
            nc.gpsimd.memset(m[:], 0.0)
            nc.gpsimd.affine_select(out=m[:], in_=m[:], compare_op=ALU.is_ge,
                                    fill=NEG, base=W2 - off, channel_multiplier=-1,
                                    pattern=[[1, KX]])
            nc.gpsimd.affine_select(out=m[:], in_=m[:], compare_op=ALU.is_ge,
                                    fill=NEG, base=W2 + off, channel_multiplier=1,
                                    pattern=[[-1, KX]])
            mask_by_off[off] = m

        qt_sb = {}

        def emit_qt_group(g):
            tiles = []
            for e in range(ND):
                pq = ps.tile([128, GQ], F32, tag=f"q{e % 2}", bufs=1,
                             name=f"qtp{g}_{e}")
                for d in range(ND):
                    nc.tensor.matmul(pq[:], wt[d][:, 128 * e:128 * (e + 1)],
                                     xtt[d][:, GQ * g:GQ * (g + 1)],
                                     start=(d == 0), stop=(d == ND - 1))
                st = pool.tile([128, GQ], BF16, tag=f"qt{e}", bufs=1,
                               name=f"qt{g}_{e}")
                if e % 2 == 0:
                    nc.vector.tensor_copy(st[:], pq[:])
                else:
                    nc.scalar.copy(st[:], pq[:])
                tiles.append(st)
            qt_sb[g] = tiles

        state_a = {}
        state_b = {}

        def emit_head(i):
            g = i // (GQ // QTL)
            qloc = (i % (GQ // QTL)) * QTL
            kx = min(max(128 * i - W2, 0), SEQ - KX)
            off = 128 * i - kx
            sp = ps.tile([128, KX], F32, tag="s", bufs=2, name=f"s{i}")
            for e in range(ND):
                nc.tensor.matmul(sp[:], qt_sb[g][e][:, qloc:qloc + QTL],
                                 xtt[e][:, kx:kx + KX],
                                 start=(e == 0), stop=(e == ND - 1))
            sm = pool.tile([128, KX], F32, tag="sm", bufs=2, name=f"sm{i}")
            nc.vector.tensor_tensor(out=sm[:], in0=sp[:], in1=mask_by_off[off][:],
                                    op=ALU.add)
            negmax = pool.tile([128, 1], F32, tag="nm", bufs=3, name=f"nm{i}")
            nc.vector.tensor_reduce(negmax[:], sm[:], axis=X, op=ALU.max, negate=True)
            probs = pool.tile([128, KX], BF16, tag="pb", bufs=2, name=f"pb{i}")
            sums = pool.tile([128, 1], F32, tag="sums", bufs=3, name=f"sums{i}")
            nc.scalar.activation(probs[:], sm[:], AF.Exp,
                                 bias=negmax[:], scale=1.0, accum_out=sums[:])
            recip = pool.tile([128, 1], F32, tag="recip", bufs=3, name=f"recip{i}")
            nc.vector.reciprocal(recip[:], sums[:])
            state_a[i] = (probs, recip)

        def emit_tail_a(i):
            probs, recip = state_a.pop(i)
            tp = ps.tile([128, KX], BF16, tag="t", bufs=2, name=f"tp{i}")
            for j in range(KX // 128):
                nc.tensor.transpose(tp[:, 128 * j:128 * (j + 1)],
                                    probs[:, 128 * j:128 * (j + 1)], idn[:])
            probsT = pool.tile([128, KX], BF16, tag="pt", bufs=2, name=f"pt{i}")
            if i % 2 == 0:
                nc.vector.tensor_copy(probsT[:], tp[:])
            else:
                nc.scalar.copy(probsT[:], tp[:])
            state_b[i] = (probsT, recip)

        def emit_tail_b(i):
            probsT, recip = state_b.pop(i)
            if i == 0:
                rhs = [xn_ap(0), xn_ap(1)]
            elif i == NQ - 1:
                rhs = [xn_ap(NQ - 2), xn_ap(NQ - 1)]
            else:
                rhs = [xs_ap(i - 1), xs_ap(i)]
            ra = ps.tile([128, HID], F32, tag="ra", bufs=1, name=f"ra{i}")
            for h in range(2):
                cols = slice(512 * h, 512 * (h + 1))
                for j in range(2):
                    nc.tensor.matmul(ra[:, cols],
                                     probsT[:, 128 * j:128 * (j + 1)],
                                     rhs[j][:, cols],
                                     start=(j == 0), stop=(j == 1))
            rr = pool.tile([128, HID], BF16, tag="rr", bufs=2, name=f"rr{i}")
            nc.scalar.activation(rr[:], ra[:], AF.Relu, bias=0.0, scale=recip[:])
            ot = pool.tile([128, HID], BF16, tag="ot", bufs=2, name=f"ot{i}")
            nc.gpsimd.tensor_tensor(out=ot[:], in0=rr[:], in1=xn_ap(i), op=ALU.add)
            nc.sync.dma_start(out[128 * i:128 * (i + 1), :], ot[:])

        emit_qt_group(0)
        for i in range(NQ + 2):
            if i < NQ:
                if i % 4 == 2 and i // 4 + 1 < NG:
                    emit_qt_group(i // 4 + 1)
                emit_head(i)
            if 1 <= i <= NQ:
                emit_tail_a(i - 1)
            if i >= 2:
                emit_tail_b(i - 2)

    return nc


def _run(x_all, W, trace=False, tmpdir=None, trace_cores=None):
    import ml_dtypes
    import concourse.bass as bass
    from concourse import bass_utils

    BF = ml_dtypes.bfloat16

    nc = bass.Bass("TRN2", target_bir_lowering=False, debug=False, num_devices=8)
    _build(nc)
    _legalize_waits(nc)

    Wb = np.ascontiguousarray(W.astype(BF))
    xb = x_all.astype(BF)
    in_maps = []
    for c in range(8):
        in_maps.append({
            "w": Wb,
            "xt": np.ascontiguousarray(xb[c].T),
            "xn": np.ascontiguousarray(xb[c]),
        })
    kwargs = {}
    if trace:
        kwargs = dict(trace=True, tmpdir=tmpdir,
                      trace_cores=trace_cores if trace_cores is not None else [0])
    res = bass_utils.run_bass_kernel_spmd(nc, in_maps, core_ids=list(range(8)),
                                          **kwargs)
    out = np.stack([r["out"] for r in res.results]).astype(np.float32)
    return out, res


def kernel(repr, W):
    x_all = np.ascontiguousarray(np.asarray(repr, dtype=np.float32))
    Wm = np.ascontiguousarray(np.asarray(W, dtype=np.float32))
    out, _ = _run(x_all, Wm, trace=False)
    return out


# Alias for external drivers that expect a `build(nc)` entry point.
build = _build


# revision 7
# speedup vs baseline: 1.1958x; 1.1958x over previous
"""Banded-attention (AttentionWindow) TRN2 kernel, data-parallel over batch on 8 NeuronCores.

Reference computation (per batch b):
  Q = x @ W;  scores = Q @ x^T;  scores[|i-j| > 64] = -1e9
  probs = softmax(scores, axis=-1);  out = x + relu(probs @ x)

Kernel strategy (v3):
  - One batch per core (batch=8, n_cores=8), W replicated. No collectives.
  - All matmul operands are bf16 (validated offline: rel_fro ~8e-3 vs the
    2e-2 gate); softmax statistics and PSUM accumulation stay fp32.
  - The DMA subsystem is PACKET-rate bound (~90 packets/us aggregate,
    one packet per partition-line per DMA, up to >=6KB per packet), so
    inputs are packed host-side for maximal bytes/packet:
      wxt [1024, 3072] = concat(W, x^T) along cols   -> 6KB lines
      xnx [2048, 2048] = concat(x, x shifted by 64)  -> 4KB lines
    The 64-row-shifted copy makes every 256-wide PV key window exactly two
    128-aligned chunks (the |i-j|<=64 band windows sit at 64-offsets).
  - PE warm-up uses real matmuls (PE transposes do not engage the HAM
    activity monitor -> clock stays throttled at 1.2 GHz), rotated over 4
    PSUM banks to dodge write-after-write serialization.
  - Projection group 0 is d-outer / e-blocked so it streams behind the
    arriving wxt tiles instead of waiting for all of them.
  - Head/tail split per tile: head(i) | transpose(i-1) | PV..out(i-2)
    keeps the PE from stalling on PSUM->SBUF copies. Mask-add and the
    softmax max fuse into one DVE tensor_tensor_reduce.

Inputs: repr [8, 2048, 1024] f32, W [1024, 1024] f32.
Output: [8, 2048, 1024] f32.
"""
from contextlib import ExitStack

import numpy as np

SEQ, HID = 2048, 1024
W2 = 64                  # window half-width
QTL = 128                # queries per softmax tile
KX = 256                 # exact key window per q-tile
NQ = SEQ // QTL          # 16
GQ = 512                 # queries per Q^T-projection group
NG = SEQ // GQ           # 4
ND = HID // 128          # 8 contraction chunks
NEG = -1e9
WARM_N = 12              # PE warm-up matmuls (fill initial DMA wait, warm HAM)


def _legalize_waits(nc):
    """This walrus build accepts 1 sync wait per instruction (2 on
    EventSemaphore). Hoist excess waits onto EventSemaphore prefixes on the
    same engine."""
    from concourse import mybir

    n = 0
    for func in nc.m.functions:
        for blk in func.blocks:
            out = []
            changed = False
            for inst in list(blk.instructions):
                si = inst.sync_info
                cap = 2 if isinstance(inst, mybir.InstEventSemaphore) else 1
                if si is not None and len(si.on_wait) > cap:
                    waits = list(si.on_wait)
                    for i in range(cap, len(waits), 2):
                        ev = mybir.InstEventSemaphore(
                            name=f"{inst.name}_waitfix{i}",
                            engine=inst.engine,
                            ins=[],
                            outs=[],
                            sync_info=mybir.SyncInfo(on_wait=waits[i:i + 2],
                                                     on_update=[]),
                        )
                        out.append(ev)
                        n += 1
                    inst.sync_info = mybir.SyncInfo(on_wait=waits[:cap],
                                                    on_update=list(si.on_update))
                    changed = True
                out.append(inst)
            if changed:
                blk.instructions = out
    return n


def _build(nc):
    import concourse.tile as tile
    from concourse import masks, mybir

    F32 = mybir.dt.float32
    BF16 = mybir.dt.bfloat16
    AF = mybir.ActivationFunctionType
    ALU = mybir.AluOpType
    X = mybir.AxisListType.X

    wxt = nc.dram_tensor("wxt", [HID, HID + SEQ], BF16, kind="ExternalInput").ap()
    xnx = nc.dram_tensor("xnx", [SEQ, 2 * HID], BF16, kind="ExternalInput").ap()
    out = nc.dram_tensor("out", [SEQ, HID], BF16, kind="ExternalOutput").ap()

    with tile.TileContext(nc) as tc, ExitStack() as ctx:
        pool = ctx.enter_context(tc.tile_pool(name="sb", bufs=1))
        ps = ctx.enter_context(tc.tile_pool(name="ps", bufs=1, space="PSUM"))

        # ---- warm-up tiles (no DMA deps): memset first so the PE can start
        # real matmuls immediately and warm the HAM clock gate.
        warm_w = pool.tile([128, 128], BF16, tag="warmw", name="warmw")
        warm_x = pool.tile([128, 512], BF16, tag="warmx", name="warmx")
        nc.gpsimd.memset(warm_w[:], 0.0)
        nc.gpsimd.memset(warm_x[:], 0.0)
        wrm = [ps.tile([128, GQ], F32, tag=f"q{t}", name=f"warmps{t}")
               for t in range(4)]
        for k in range(WARM_N):
            nc.tensor.matmul(wrm[k % 4][:], warm_w[:], warm_x[:],
                             start=True, stop=True)

        # ---- resident inputs (packed): wxtt[d] = [W rows | x^T rows] chunk d;
        # xnxt[k] = [x tile k | x tile shifted by 64 rows].
        wxtt = [pool.tile([128, HID + SEQ], BF16, tag=f"wx{d}", name=f"wx{d}")
                for d in range(ND)]
        xnxt = [pool.tile([128, 2 * HID], BF16, tag=f"xn{k}", name=f"xn{k}")
                for k in range(NQ)]

        def wt(d):
            return wxtt[d][:, 0:HID]

        def xtt(d):
            return wxtt[d][:, HID:HID + SEQ]

        def xn_ap(k):
            return xnxt[k][:, 0:HID]

        def xs_ap(j):
            return xnxt[j][:, HID:2 * HID]

        for d in range(ND):
            nc.sync.dma_start(wxtt[d][:], wxt[128 * d:128 * (d + 1), :])
        for k in range(NQ):
            nc.sync.dma_start(xnxt[k][:], xnx[128 * k:128 * (k + 1), :])

        # ---- identity (bf16, for probs transposes) + banded masks
        idn = pool.tile([128, 128], BF16, tag="idn", name="idn")
        masks.make_identity(nc, idn[:])
        mask_by_off = {}
        for off in (0, 64, 128):
            m = pool.tile([128, KX], F32, tag=f"mask{off}", name=f"mask{off}")
            nc.gpsimd.memset(m[:], 0.0)
            nc.gpsimd.affine_select(out=m[:], in_=m[:], compare_op=ALU.is_ge,
                                    fill=NEG, base=W2 - off, channel_multiplier=-1,
                                    pattern=[[1, KX]])
            nc.gpsimd.affine_select(out=m[:], in_=m[:], compare_op=ALU.is_ge,
                                    fill=NEG, base=W2 + off, channel_multiplier=1,
                                    pattern=[[-1, KX]])
            mask_by_off[off] = m

        qt_sb = {}

        def emit_qt_group0():
            """Group 0, d-outer / e-blocked: consumes wxtt[d] as each DMA
            lands instead of waiting for all of them. Uses 4 PSUM banks."""
            tiles = [None] * ND
            for eb in range(2):
                pqs = [ps.tile([128, GQ], F32, tag=f"q{e % 4}", bufs=1,
                               name=f"qtp0_{4 * eb + e}") for e in range(4)]
                for d in range(ND):
                    for e in range(4):
                        ee = 4 * eb + e
                        nc.tensor.matmul(pqs[e][:], wt(d)[:, 128 * ee:128 * (ee + 1)],
                                         xtt(d)[:, 0:GQ],
                                         start=(d == 0), stop=(d == ND - 1))
                for e in range(4):
                    ee = 4 * eb + e
                    st = pool.tile([128, GQ], BF16, tag=f"qt{ee}", bufs=1,
                                   name=f"qt0_{ee}")
                    if e % 2 == 0:
                        nc.vector.tensor_copy(st[:], pqs[e][:])
                    else:
                        nc.scalar.copy(st[:], pqs[e][:])
                    tiles[ee] = st
            qt_sb[0] = tiles

        def emit_qt_group(g):
            tiles = []
            for e in range(ND):
                pq = ps.tile([128, GQ], F32, tag=f"q{e % 4}", bufs=1,
                             name=f"qtp{g}_{e}")
                for d in range(ND):
                    nc.tensor.matmul(pq[:], wt(d)[:, 128 * e:128 * (e + 1)],
                                     xtt(d)[:, GQ * g:GQ * (g + 1)],
                                     start=(d == 0), stop=(d == ND - 1))
                st = pool.tile([128, GQ], BF16, tag=f"qt{e}", bufs=1,
                               name=f"qt{g}_{e}")
                if e % 2 == 0:
                    nc.vector.tensor_copy(st[:], pq[:])
                else:
                    nc.scalar.copy(st[:], pq[:])
                tiles.append(st)
            qt_sb[g] = tiles

        state_a = {}
        state_b = {}

        def emit_head(i):
            g = i // (GQ // QTL)
            qloc = (i % (GQ // QTL)) * QTL
            kx = min(max(128 * i - W2, 0), SEQ - KX)
            off = 128 * i - kx
            sp = ps.tile([128, KX], F32, tag="s", bufs=1, name=f"s{i}")
            for e in range(ND):
                nc.tensor.matmul(sp[:], qt_sb[g][e][:, qloc:qloc + QTL],
                                 xtt(e)[:, kx:kx + KX],
                                 start=(e == 0), stop=(e == ND - 1))
            sm = pool.tile([128, KX], F32, tag="sm", bufs=2, name=f"sm{i}")
            nc.vector.tensor_tensor(out=sm[:], in0=sp[:], in1=mask_by_off[off][:],
                                    op=ALU.add)
            negmax = pool.tile([128, 1], F32, tag="nm", bufs=3, name=f"nm{i}")
            nc.vector.tensor_reduce(negmax[:], sm[:], axis=X, op=ALU.max,
                                    negate=True)
            probs = pool.tile([128, KX], BF16, tag="pb", bufs=2, name=f"pb{i}")
            sums = pool.tile([128, 1], F32, tag="sums", bufs=3, name=f"sums{i}")
            nc.scalar.activation(probs[:], sm[:], AF.Exp,
                                 bias=negmax[:], scale=1.0, accum_out=sums[:])
            recip = pool.tile([128, 1], F32, tag="recip", bufs=3, name=f"recip{i}")
            nc.vector.reciprocal(recip[:], sums[:])
            state_a[i] = (probs, recip)

        def emit_tail_a(i):
            probs, recip = state_a.pop(i)
            tp = ps.tile([128, KX], BF16, tag="t", bufs=1, name=f"tp{i}")
            for j in range(KX // 128):
                nc.tensor.transpose(tp[:, 128 * j:128 * (j + 1)],
                                    probs[:, 128 * j:128 * (j + 1)], idn[:])
            probsT = pool.tile([128, KX], BF16, tag="pt", bufs=2, name=f"pt{i}")
            if i % 2 == 0:
                nc.vector.tensor_copy(probsT[:], tp[:])
            else:
                nc.scalar.copy(probsT[:], tp[:])
            state_b[i] = (probsT, recip)

        def emit_tail_b(i):
            probsT, recip = state_b.pop(i)
            if i == 0:
                rhs = [xn_ap(0), xn_ap(1)]
            elif i == NQ - 1:
                rhs = [xn_ap(NQ - 2), xn_ap(NQ - 1)]
            else:
                rhs = [xs_ap(i - 1), xs_ap(i)]
            ra = ps.tile([128, HID], F32, tag="ra", bufs=1, name=f"ra{i}")
            for h in range(2):
                cols = slice(512 * h, 512 * (h + 1))
                for j in range(2):
                    nc.tensor.matmul(ra[:, cols],
                                     probsT[:, 128 * j:128 * (j + 1)],
                                     rhs[j][:, cols],
                                     start=(j == 0), stop=(j == 1))
            rr = pool.tile([128, HID], BF16, tag="rr", bufs=2, name=f"rr{i}")
            nc.scalar.activation(rr[:], ra[:], AF.Relu, bias=0.0, scale=recip[:])
            ot = pool.tile([128, HID], BF16, tag="ot", bufs=2, name=f"ot{i}")
            # last tiles: vector engine is idle at the drain and ~2x faster here
            eng = nc.vector if i >= NQ - 2 else nc.gpsimd
            eng.tensor_tensor(out=ot[:], in0=rr[:], in1=xn_ap(i), op=ALU.add)
            nc.sync.dma_start(out[128 * i:128 * (i + 1), :], ot[:])

        emit_qt_group0()
        for i in range(NQ + 2):
            if i < NQ:
                if i % 4 == 2 and i // 4 + 1 < NG:
                    emit_qt_group(i // 4 + 1)
                emit_head(i)
            if 1 <= i <= NQ:
                emit_tail_a(i - 1)
            if i >= 2:
                emit_tail_b(i - 2)

    return nc


def _run(x_all, W, trace=False, tmpdir=None, trace_cores=None):
    import ml_dtypes
    import concourse.bass as bass
    from concourse import bass_utils

    BF = ml_dtypes.bfloat16

    nc = bass.Bass("TRN2", target_bir_lowering=False, debug=False, num_devices=8)
    _build(nc)
    _legalize_waits(nc)

    Wb = W.astype(BF)
    xb = x_all.astype(BF)
    in_maps = []
    for c in range(8):
        xs = np.concatenate([xb[c][64:], np.zeros((64, HID), dtype=BF)], axis=0)
        in_maps.append({
            "wxt": np.ascontiguousarray(np.concatenate([Wb, xb[c].T], axis=1)),
            "xnx": np.ascontiguousarray(np.concatenate([xb[c], xs], axis=1)),
        })
    kwargs = {}
    if trace:
        kwargs = dict(trace=True, tmpdir=tmpdir,
                      trace_cores=trace_cores if trace_cores is not None else [0])
    res = bass_utils.run_bass_kernel_spmd(nc, in_maps, core_ids=list(range(8)),
                                          **kwargs)
    out = np.stack([r["out"] for r in res.results]).astype(np.float32)
    return out, res


def kernel(repr, W):
    x_all = np.ascontiguousarray(np.asarray(repr, dtype=np.float32))
    Wm = np.ascontiguousarray(np.asarray(W, dtype=np.float32))
    out, _ = _run(x_all, Wm, trace=False)
    return out


# Alias for external drivers that expect a `build(nc)` entry point.
build = _build
